# revision 1
# baseline (speedup 1.0000x reference)
"""MMoE-style CustomizedGateControl kernel for 8x TRN2 NeuronCores.

Data-parallel over the batch dim (16384 -> 8 x 2048). Per core:
  - 12 expert GEMMs ([2048,512]@[512,256]) + per-task gates fused as one
    wide f32r matmul sweep with batch rows on PSUM partitions
  - bias-add (DVE) + ReLU (ACT) drain to fp16 SBUF
  - gated combine fused with the [b,e]->[e,b] transpose as fp16 PE matmuls:
    info_t.T = sum_g X_g.T @ diag(gate_tg), diag built by one DVE
    tensor_scalar (identity * per-partition gate column)
  - tower MLP GEMMs in f32r
All parameters replicated; no collectives.
"""

import sys

if "/opt/trn_rl_repo" not in sys.path:
    sys.path.insert(0, "/opt/trn_rl_repo")

import numpy as np

import concourse.bacc as bacc
import concourse.mybir as mybir
import concourse.tile as tile
from concourse.bass_utils import run_bass_kernel_spmd

# problem dims
B, D, E, H = 16384, 512, 256, 128
S, K, T = 4, 4, 2
NCORES = 8
BC = B // NCORES          # 2048 batch rows per core
P = 128                   # partitions
NB = BC // P              # 16 b-tiles per core
NE = S + T * K            # 12 experts
G = S + K                 # 8 gate inputs per task
WCOLS = NE * E            # 3072 expert output columns
WALL = WCOLS + T * G      # 3088 = experts + gate columns

f32 = mybir.dt.float32
f32r = mybir.dt.float32r
f16 = mybir.dt.float16


def _expert_col(t: int, j: int) -> int:
    """Column offset in the fused expert output for gate input j of task t."""
    if j < S:
        return j * E                      # shared expert j
    return (S + t * K + (j - S)) * E      # task expert (t, j-S)


def _build():
    nc = bacc.Bacc("TRN2", target_bir_lowering=False, debug=False)

    xt_d = nc.dram_tensor("xt", [D, BC], f16, kind="ExternalInput").ap()
    wall_d = nc.dram_tensor("wall", [D, WALL], f16, kind="ExternalInput").ap()
    biasb_d = nc.dram_tensor("biasb", [P, WCOLS], f16, kind="ExternalInput").ap()
    tw1_d = nc.dram_tensor("tw1", [T, E, H], f16, kind="ExternalInput").ap()
    tb1_d = nc.dram_tensor("tb1", [H, T], f32, kind="ExternalInput").ap()
    tw2_d = nc.dram_tensor("tw2", [H, T], f16, kind="ExternalInput").ap()
    ident_d = nc.dram_tensor("ident", [P, P], f16, kind="ExternalInput").ap()
    out_d = nc.dram_tensor("out", [T, BC], f32, kind="ExternalOutput").ap()

    KC = D // P  # 4 contraction chunks
    HW = 512  # columns per psum chunk (1 bank)

    with tile.TileContext(nc) as tc:
        with (
            tc.tile_pool(name="const", bufs=1) as const,
            tc.tile_pool(name="dg", bufs=2) as dg_pool,
            tc.tile_pool(name="hsb", bufs=2) as hsb_pool,
        ):
            # ---- persistent inputs (critical chunks first: first expert MM
            # needs xt[k][:,0:128] + wall[k][:,0:1024]) ----
            xt_t = [const.tile([P, BC], f16, tag=f"xt{k}", name=f"xt{k}") for k in range(KC)]
            wall_t = [const.tile([P, WALL], f16, tag=f"wall{k}", name=f"wall{k}") for k in range(KC)]
            biasb = const.tile([P, WCOLS], f16, tag="biasb", name="biasb")
            ident = const.tile([P, P], f16, tag="ident", name="ident")
            for k in range(KC):
                rs = slice(k * P, (k + 1) * P)
                nc.sync.dma_start(xt_t[k][:, 0:P], xt_d[rs, 0:P])
                nc.gpsimd.dma_start(wall_t[k][:, 0:1024], wall_d[rs, 0:1024])
            nc.sync.dma_start(biasb[:, 0:1024], biasb_d[:, 0:1024])
            for k in range(KC):
                rs = slice(k * P, (k + 1) * P)
                nc.sync.dma_start(xt_t[k][:, P : BC // 2], xt_d[rs, P : BC // 2])
                nc.gpsimd.dma_start(wall_t[k][:, 1024:2048], wall_d[rs, 1024:2048])
            nc.scalar.dma_start(biasb[:, 1024:2048], biasb_d[:, 1024:2048])
            for k in range(KC):
                rs = slice(k * P, (k + 1) * P)
                nc.sync.dma_start(xt_t[k][:, BC // 2 : BC], xt_d[rs, BC // 2 : BC])
                nc.gpsimd.dma_start(wall_t[k][:, 2048:3072], wall_d[rs, 2048:3072])
            nc.scalar.dma_start(biasb[:, 2048:3072], biasb_d[:, 2048:3072])
            nc.sync.dma_start(ident[:], ident_d[:])
            for k in range(KC):
                rs = slice(k * P, (k + 1) * P)
                nc.gpsimd.dma_start(wall_t[k][:, WCOLS:WALL], wall_d[rs, WCOLS:WALL])
            tw1_t = {}
            tw1_t = {}
            for t in range(T):
                for kc in range(2):
                    t_ = const.tile([P, H], f16, tag=f"tw1_{t}_{kc}", name=f"tw1_{t}_{kc}")
                    nc.sync.dma_start(t_[:], tw1_d[t, kc * P : (kc + 1) * P, :])
                    tw1_t[(t, kc)] = t_
            tb1 = const.tile([H, T], f32, tag="tb1", name="tb1")
            nc.sync.dma_start(tb1[:], tb1_d[:])
            tw2 = const.tile([H, T], f16, tag="tw2", name="tw2")
            nc.sync.dma_start(tw2[:], tw2_d[:])
            infoT = []  # [e-chunk on partitions, full-batch free] per (t, ec)
            for t in range(T):
                for ec in range(2):
                    infoT.append(
                        const.tile([P, BC], f16, tag=f"infoT{t}_{ec}", name=f"infoT{t}_{ec}")
                    )
            out_sb = const.tile([1, T * BC], f32, tag="out_sb", name="out_sb")

            with (
                tc.tile_pool(name="expps", bufs=4, space="PSUM") as expps_pool,
                tc.tile_pool(name="gateps", bufs=1, space="PSUM") as gateps_pool,
                tc.tile_pool(name="ctps", bufs=3, space="PSUM") as ctps_pool,
            ):
                gate_ps = gateps_pool.tile([P, NB * T * G], f32, tag="gateps", name="gateps")
                exp_sb_t = [
                    const.tile([P, WCOLS], f16, tag=f"expsb{i}", name=f"expsb{i}")
                    for i in range(NB)
                ]
                gsb_t = [
                    const.tile([P, T * G], f16, tag=f"gsb{i}", name=f"gsb{i}")
                    for i in range(NB)
                ]

                # chunk-major expert sweep: all b-tiles for one 512-col chunk
                # before the next, so compute saturates while weights stream in
                for third in range(WCOLS // HW):
                    c0 = third * HW
                    for i in range(NB):
                        bs = slice(i * P, (i + 1) * P)
                        exp_sb = exp_sb_t[i]
                        pe = expps_pool.tile([P, HW], f32, tag="expps", name="expps")
                        for k in range(KC):
                            nc.tensor.matmul(
                                pe[:],
                                xt_t[k][:, bs],
                                wall_t[k][:, c0 : c0 + HW],
                                start=(k == 0),
                                stop=(k == KC - 1),
                            )
                        nc.vector.tensor_add(
                            exp_sb[:, c0 : c0 + HW], pe[:], biasb[:, c0 : c0 + HW]
                        )
                        nc.scalar.activation(
                            exp_sb[:, c0 : c0 + HW],
                            exp_sb[:, c0 : c0 + HW],
                            mybir.ActivationFunctionType.Relu,
                        )
                    if third == 0:
                        for i in range(NB):
                            bs = slice(i * P, (i + 1) * P)
                            gsl = slice(i * T * G, (i + 1) * T * G)
                            for k in range(KC):
                                nc.tensor.matmul(
                                    gate_ps[:, gsl],
                                    xt_t[k][:, bs],
                                    wall_t[k][:, WCOLS:WALL],
                                    start=(k == 0),
                                    stop=(k == KC - 1),
                                )
                            nc.scalar.copy(gsb_t[i][:], gate_ps[:, gsl])

                # combine+transpose sweep
                for i in range(NB):
                    bs = slice(i * P, (i + 1) * P)
                    exp_sb = exp_sb_t[i]
                    diag = dg_pool.tile([P, T * G * P], f16, tag="dg", name="dg")
                    nc.vector.tensor_mul(
                        diag[:].rearrange("p (j c) -> p j c", c=P),
                        ident[:, None, :].broadcast_to([P, T * G, P]),
                        gsb_t[i][:, :, None].broadcast_to([P, T * G, P]),
                    )
                    for ec in range(2):
                        ct = ctps_pool.tile([P, T * P], f32, tag="ctps", name="ctps")
                        for g in range(S):
                            c = _expert_col(0, g)
                            nc.tensor.matmul(
                                ct[:],
                                exp_sb[:, c + ec * P : c + (ec + 1) * P],
                                diag[:, g * 2 * P : (g * 2 + 2) * P],
                                start=(g == 0),
                                stop=False,
                                skip_group_check=True,
                            )
                        for t in range(T):
                            for g in range(S, G):
                                c = _expert_col(t, g)
                                nc.tensor.matmul(
                                    ct[:, t * P : (t + 1) * P],
                                    exp_sb[:, c + ec * P : c + (ec + 1) * P],
                                    diag[:, (g * 2 + t) * P : (g * 2 + t + 1) * P],
                                    start=False,
                                    stop=(g == G - 1),
                                    skip_group_check=True,
                                )
                        for t in range(T):
                            nc.scalar.copy(
                                infoT[t * 2 + ec][:, bs], ct[:, t * P : (t + 1) * P]
                            )

            # towers
            with (
                tc.tile_pool(name="hps", bufs=2, space="PSUM") as hps_pool,
                tc.tile_pool(name="ops", bufs=2, space="PSUM") as ops_pool,
            ):
                for t in range(T):
                    for bc in range(BC // 512):
                        cs = slice(bc * 512, (bc + 1) * 512)
                        hp = hps_pool.tile([P, 512], f32, tag="hps", name="hps")
                        for kc in range(2):
                            nc.tensor.matmul(
                                hp[:],
                                tw1_t[(t, kc)][:],
                                infoT[t * 2 + kc][:, cs],
                                start=(kc == 0),
                                stop=(kc == 1),
                            )
                        hs = hsb_pool.tile([P, 512], f16, tag="hsb", name="hsb")
                        nc.scalar.activation(
                            hs[:],
                            hp[:],
                            mybir.ActivationFunctionType.Relu,
                            bias=tb1[:, t : t + 1],
                        )
                        op = ops_pool.tile([1, 512], f32, tag="ops", name="ops")
                        nc.tensor.matmul(
                            op[:],
                            tw2[:, t : t + 1],
                            hs[:],
                            start=True,
                            stop=True,
                        )
                        r = t * (BC // 512) + bc
                        nc.vector.tensor_copy(
                            out_sb[0:1, r * 512 : (r + 1) * 512], op[0:1, :]
                        )
                        nc.sync.dma_start(
                            out_d.rearrange("t n -> (t n)")[
                                None, r * 512 : (r + 1) * 512
                            ],
                            out_sb[0:1, r * 512 : (r + 1) * 512],
                        )

    nc.compile()
    return nc


_NC = None


def _get_nc():
    global _NC
    if _NC is None:
        _NC = _build()
    return _NC


def _prep_shared(shared_W, shared_b, task_W, task_b, gate_W, tower_W1, tower_b1, tower_W2):
    cols = [np.asarray(shared_W[s]) for s in range(S)]
    cols += [np.asarray(task_W[t, k]) for t in range(T) for k in range(K)]
    gwi = np.empty((D, T * G), np.float32)
    for t in range(T):
        gwi[:, t::T] = np.asarray(gate_W[t])  # column g*T+t = gate (t, g)
    cols += [gwi]
    wall = np.ascontiguousarray(np.concatenate(cols, axis=1), dtype=np.float16)
    bias_all = np.concatenate(
        [np.asarray(shared_b).reshape(-1), np.asarray(task_b).reshape(-1)]
    ).astype(np.float32)
    biasb = np.ascontiguousarray(np.broadcast_to(bias_all, (P, WCOLS)).astype(np.float16))
    tw1 = np.ascontiguousarray(tower_W1, dtype=np.float16)
    tb1 = np.ascontiguousarray(np.asarray(tower_b1).T, dtype=np.float32)   # [H, T]
    tw2 = np.ascontiguousarray(np.asarray(tower_W2)[:, :, 0].T, dtype=np.float16)  # [H, T]
    ident = np.eye(P, dtype=np.float16)
    return wall, biasb, tw1, tb1, tw2, ident


def kernel(
    x,
    shared_W,
    shared_b,
    task_W,
    task_b,
    gate_W,
    tower_W1,
    tower_b1,
    tower_W2,
    tower_b2,
    _trace=False,
    _tmpdir=None,
):
    nc = _get_nc()
    x = np.asarray(x, dtype=np.float32)
    wall, biasb, tw1, tb1, tw2, ident = _prep_shared(
        shared_W, shared_b, task_W, task_b, gate_W, tower_W1, tower_b1, tower_W2
    )
    in_maps = []
    for c in range(NCORES):
        xt = np.ascontiguousarray(x[c * BC : (c + 1) * BC, :].T.astype(np.float16))
        in_maps.append(
            {
                "xt": xt,
                "wall": wall,
                "biasb": biasb,
                "tw1": tw1,
                "tb1": tb1,
                "tw2": tw2,
                "ident": ident,
            }
        )
    kw = {}
    if _trace:
        kw = {"trace": True, "tmpdir": _tmpdir}
    res = run_bass_kernel_spmd(nc, in_maps, core_ids=list(range(NCORES)), **kw)
    out = np.concatenate([res.results[c]["out"] for c in range(NCORES)], axis=1)
    out = out + np.asarray(tower_b2, dtype=np.float32)[:, 0][:, None]
    result = out[:, :, None].astype(np.float32)  # [T, B, 1]
    if _trace:
        return result, res
    return result

